# revision 9
# baseline (speedup 1.0000x reference)
"""Trainium2 Bass kernel v2 for the decoder block (LN->attn->res->LN->FFN->res).

Sharding: 8 NeuronCores. Core c owns tokens [256c, 256(c+1)) of BOTH batches
for the token-parallel phases (LN1/QKV/FFN), and owns HEADS {2c, 2c+1} of
both batches for attention (head-parallel). Head-parallel attention makes
causal skipping SPMD-uniform: every core runs the identical
(batch, i-group, j-chunk) loop over the causal lower block-triangle, which
cuts attention matmul AND exp work to ~62% of dense.

Data movement (single 8-core AllToAlls):
  - A2A(qk): locally projected qT/kT head-chunk shards -> per-core full-T
    q/k (both batches) for its 2 heads.
  - A2A(v):  v_aug (token-major, per-head ones column for the softmax
    denominator) -> per-core full-T v for its 2 heads.
  - A2A(attn): normalized attention output (token-major bf16) -> back to
    token-parallel for residual + FFN.

Scores follow the reference convention scores[i,j] = k_i . q_j with softmax
over j (q/v are the "context" side). Causal masking costs one inline 128x128
triangular mask applied only to diagonal j-chunks; off-diagonal chunks skip
the mask multiply, and the masked-out i-columns of diagonal chunks are
skipped in the score matmul, exp, and AV matmul.

Shapes (hardcoded): B=2, T=2048, C=1024, H=16, D=64, F=4096.
"""

import sys

sys.path.insert(0, "/opt/trn_rl_repo")

import numpy as np
import ml_dtypes

import concourse.bass as bass
import concourse.bacc as bacc
import concourse.tile as tile
from concourse import mybir
from concourse.bass_utils import run_bass_kernel_spmd
from concourse.masks import make_identity

F32 = mybir.dt.float32
BF16 = mybir.dt.bfloat16
AF = mybir.ActivationFunctionType
OP = mybir.AluOpType

B, T, C = 2, 2048, 1024
H, D = 16, 64
F = 4 * C
EPS = 1e-3
N_CORES = 8
GROUPS = [[0, 1, 2, 3, 4, 5, 6, 7]]
BT = 256          # tokens per core per batch
RT = 512          # tokens per core (2 batches x 256)
NT = RT // 128    # 4 local token chunks of 128: chunk = 2*bb + tc
NC_F = C // 128   # 8 feature chunks
NF_F = F // 128   # 32 ffn feature chunks
NIG = 4           # i-groups of 512 across a batch's T (attention phase)
NJC = T // 128    # 16 j-chunks of 128 across a batch's T
SCALE = 1.0 / float(np.sqrt(D))

# A2A shard sizes (elems, bf16)
QK_SH = 2 * 128 * RT          # {qT,kT} chunk g: [128, 512] each
V_SH = NT * 2 * 65 * 128      # v_aug[:, :, 2g:2g+2, :]
AT_SH = 2 * 2 * 2 * 64 * 128  # attn_all[:, :, 2g:2g+2, :, :]


def _mask4():
    # mask4[d][r, i] = 1 if token 128d + r is visible to i-column i (j <= i)
    d = np.arange(4)[:, None, None]
    j = np.arange(128)[None, :, None]
    i = np.arange(RT)[None, None, :]
    return (128 * d + j <= i).astype(ml_dtypes.bfloat16)


def build_nc(reps=1, affine=True, inline=None):
    nc = bacc.Bacc(None, target_bir_lowering=False)

    def in_tensor(name, shape, dt):
        if inline is not None and name in inline:
            return nc.inline_tensor(inline[name], name=name)
        return nc.dram_tensor(name, shape, dt, kind="ExternalInput")

    x_d = in_tensor("x_loc", [RT, C], F32)
    wq_d = in_tensor("Wq", [C, C], BF16)
    wk_d = in_tensor("Wk", [C, C], BF16)
    wv_d = in_tensor("Wv", [C, C], BF16)
    w1_d = in_tensor("W1", [C, F], BF16)
    w2_d = in_tensor("W2", [F, C], BF16)
    bq_d = in_tensor("bq", [C], F32)
    bk_d = in_tensor("bk", [C], F32)
    bv_d = in_tensor("bv", [C], F32)
    b1_d = in_tensor("b1", [F], F32)
    b2_d = in_tensor("b2", [C], F32)
    g1_d = in_tensor("ln1_g", [C], F32)
    be1_d = in_tensor("ln1_b", [C], F32)
    g2_d = in_tensor("ln2_g", [C], F32)
    be2_d = in_tensor("ln2_b", [C], F32)
    y_d = nc.dram_tensor("y", [RT, C], F32, kind="ExternalOutput")

    mask_d = nc.inline_tensor(_mask4(), name="mask4")

    cc_bufs = [
        (nc.dram_tensor(f"qk_in{r}", [8 * QK_SH], BF16),
         nc.dram_tensor(f"qk_out{r}", [8 * QK_SH], BF16),
         nc.dram_tensor(f"v_in{r}", [8 * V_SH], BF16),
         nc.dram_tensor(f"v_out{r}", [8 * V_SH], BF16),
         nc.dram_tensor(f"at_in{r}", [8 * AT_SH], BF16),
         nc.dram_tensor(f"at_out{r}", [8 * AT_SH], BF16))
        for r in range(reps)
    ]

    def bcast_row(dram_vec):
        # [n] dram vector -> [128, n] broadcast AP (partition step 0)
        return bass.AP(tensor=dram_vec.tensor, offset=dram_vec.offset,
                       ap=[[0, 128], dram_vec.ap[0]])

    with tile.TileContext(nc) as tc:
        with (
            tc.tile_pool(name="const", bufs=1) as const,
            tc.tile_pool(name="big", bufs=1) as big,
            tc.tile_pool(name="wpool", bufs=1) as wpool,
            tc.tile_pool(name="w1s", bufs=3) as w1s,
            tc.tile_pool(name="stats", bufs=4) as stats,
            tc.tile_pool(name="exs", bufs=10) as exs,
            tc.tile_pool(name="avs_pool", bufs=2) as avs_pool,
            tc.tile_pool(name="mm_ps", bufs=2, space="PSUM") as mm_ps,
            tc.tile_pool(name="st_ps", bufs=2, space="PSUM") as st_ps,
            tc.tile_pool(name="av_ps", bufs=2, space="PSUM") as av_ps,
        ):
            # ---- load x first (per chunk) so LN1 starts immediately ----
            x_first = big.tile([128, NT, C], F32, tag="x", name="x_first")
            for t in range(NT):
                nc.sync.dma_start(out=x_first[:, t],
                                  in_=x_d.rearrange("(t p) c -> p t c", p=128)[:, t])

            # ---- constants ----
            ident_b = const.tile([128, 128], BF16, tag="ident_b")
            make_identity(nc, ident_b)
            ident_f = const.tile([128, 128], F32, tag="ident_f")
            make_identity(nc, ident_f)
            eps_t = const.tile([128, 1], F32, tag="eps")
            nc.vector.memset(eps_t, EPS)
            mask_sb = const.tile([128, 4, RT], BF16, tag="mask")
            nc.sync.dma_start(out=mask_sb, in_=mask_d.rearrange("d p i -> p d i"))
            # ACT tables: Sqrt/Exp/Gelu live in three different table sets, so
            # each function switch costs a ~2.7us table load. Warm Sqrt now
            # (LN1 is first); later phases pre-warm their set with a dummy
            # call placed where the load hides under other work.
            warm_t = const.tile([128, 1], F32, tag="warm")
            nc.vector.memset(warm_t, 1.0)

            def warm(func, dep=None):
                # dep pins the dummy after a producing phase so the scheduler
                # cannot float the table switch into the middle of another
                # phase's activation stream
                w_o = stats.tile([128, 1], F32, tag="rs", name=f"warm{warm.n}")
                warm.n += 1
                if dep is None:
                    nc.scalar.activation(out=w_o, in_=warm_t, func=func)
                else:
                    # scale=0 + eps bias: safe input for any func, still
                    # data-dependent on `dep` for ordering
                    nc.scalar.activation(out=w_o, in_=dep, func=func,
                                         scale=0.0, bias=eps_t)
            warm.n = 0
            warm(AF.Sqrt)


            g1_r = g2_r = bv_r = b2_r = None
            if affine:
                g1_r = const.tile([128, 2, C], F32, tag="lnr")
                nc.sync.dma_start(out=g1_r[:, 0], in_=bcast_row(g1_d[:]))
                nc.sync.dma_start(out=g1_r[:, 1], in_=bcast_row(be1_d[:]))
                bv_r = const.tile([128, C], F32, tag="rowr")
                nc.sync.dma_start(out=bv_r, in_=bcast_row(bv_d[:]))
                g2_r = const.tile([128, 2, C], F32, tag="lnr2")
                nc.sync.dma_start(out=g2_r[:, 0], in_=bcast_row(g2_d[:]))
                nc.sync.dma_start(out=g2_r[:, 1], in_=bcast_row(be2_d[:]))
                b2_r = const.tile([128, C], F32, tag="rowr2")
                nc.sync.dma_start(out=b2_r, in_=bcast_row(b2_d[:]))

            bq_t = const.tile([128, NC_F], F32, tag="bq")
            nc.sync.dma_start(out=bq_t, in_=bq_d.rearrange("(a p) -> p a", p=128))
            bk_t = const.tile([128, NC_F], F32, tag="bk")
            nc.sync.dma_start(out=bk_t, in_=bk_d.rearrange("(a p) -> p a", p=128))
            b1_t = const.tile([128, NF_F], F32, tag="b1")
            nc.sync.dma_start(out=b1_t, in_=b1_d.rearrange("(a p) -> p a", p=128))

            # ---- body (repeated `reps` times for benchmarking) ----
            for _rep in range(reps):
              qk_in, qk_out, v_in, v_out, at_in, at_out = cc_bufs[_rep]
              if _rep == 0:
                  x_sb = x_first
              else:
                  x_sb = big.tile([128, NT, C], F32, tag="x")
                  nc.sync.dma_start(out=x_sb, in_=x_d.rearrange("(t p) c -> p t c", p=128))

              def layernorm(src_tile, h_out, g_pair):
                  # src [128, C] f32 -> h_out [128, C] bf16
                  st6 = stats.tile([128, 2, 6], F32, tag="bnst")
                  for s in range(2):
                      nc.vector.bn_stats(out=st6[:, s], in_=src_tile[:, 512 * s:512 * (s + 1)])
                  mv = stats.tile([128, 2], F32, tag="bnmv")
                  nc.vector.bn_aggr(out=mv, in_=st6)
                  rs = stats.tile([128, 1], F32, tag="rs")
                  nc.scalar.activation(out=rs, in_=mv[:, 1:2], func=AF.Sqrt, bias=eps_t)
                  nc.vector.reciprocal(out=rs, in_=rs)
                  nc.vector.tensor_scalar(out=h_out, in0=src_tile, scalar1=mv[:, 0:1],
                                          scalar2=rs, op0=OP.subtract, op1=OP.mult)
                  if affine:
                      nc.vector.tensor_mul(out=h_out, in0=h_out, in1=g_pair[:, 0])
                      nc.vector.tensor_add(out=h_out, in0=h_out, in1=g_pair[:, 1])

              # ---- LN1 -> h1 bf16, transpose -> h1T ----
              h1 = big.tile([128, NT, C], BF16, tag="h_row")
              for t in range(NT):
                  layernorm(x_sb[:, t], h1[:, t], g1_r)
              h1T = big.tile([128, NC_F, RT], BF16, tag="hT")
              for t in range(NT):
                  for fc in range(NC_F):
                      pt = mm_ps.tile([128, 128], BF16, tag="mm")
                      nc.tensor.transpose(pt, h1[:, t, 128 * fc:128 * (fc + 1)], ident_b)
                      nc.vector.tensor_copy(out=h1T[:, fc, 128 * t:128 * (t + 1)], in_=pt)
              # pre-load the Exp table set; hides under the QKV matmuls
              warm(AF.Exp, dep=h1[:, NT - 1, 0:1])

              # ---- local projections (q, k, then v) ----
              qT = big.tile([128, NC_F, RT], BF16, tag="qT")
              kT = big.tile([128, NC_F, RT], BF16, tag="kT")

              def proj(w_d_, b_t, outT):
                  w_view = w_d_.rearrange("(a p) c -> p a c", p=128)
                  for co in range(NC_F):
                      w_t = w1s.tile([128, NC_F, 128], BF16, tag="w1t")
                      nc.sync.dma_start(out=w_t, in_=w_view[:, :, 128 * co:128 * (co + 1)])
                      ps = mm_ps.tile([128, RT], F32, tag="mm")
                      for ci in range(NC_F):
                          nc.tensor.matmul(ps, w_t[:, ci, :],
                                           h1T[:, ci, :], start=(ci == 0), stop=(ci == NC_F - 1))
                      nc.scalar.activation(out=outT[:, co, :], in_=ps, func=AF.Identity,
                                           bias=b_t[:, co:co + 1])

              proj(wq_d, bq_t, qT)
              proj(wk_d, bk_t, kT)

              # A2A(qk): shard g = {qT chunk g, kT chunk g} (heads 2g, 2g+1)
              qk_in_v = qk_in[:].rearrange("(g x p t) -> p g x t", p=128, t=RT, g=8)
              for g in range(8):
                  nc.sync.dma_start(out=qk_in_v[:, g, 0], in_=qT[:, g])
                  nc.sync.dma_start(out=qk_in_v[:, g, 1], in_=kT[:, g])
              nc.gpsimd.collective_compute(
                  "AllToAll", OP.bypass, replica_groups=GROUPS,
                  ins=[qk_in[:]], outs=[qk_out[:]])

              # ---- v projection (row-major with interleaved ones column) ----
              v_aug = big.tile([128, NT, H, 65], BF16, tag="v_aug")
              nc.vector.memset(v_aug[:, :, :, 64:65], 1.0)
              wv_view = wv_d.rearrange("(a p) c -> p a c", p=128)
              for half in range(2):
                  wv_t = wpool.tile([128, NC_F, RT], BF16, tag="wvh", bufs=2)
                  nc.sync.dma_start(out=wv_t, in_=wv_view[:, :, 512 * half:512 * (half + 1)])
                  for t in range(NT):
                      ps = mm_ps.tile([128, RT], F32, tag="mm")
                      for ci in range(NC_F):
                          nc.tensor.matmul(ps, h1T[:, ci, 128 * t:128 * (t + 1)],
                                           wv_t[:, ci, :],
                                           start=(ci == 0), stop=(ci == NC_F - 1))
                      if affine:
                          nc.vector.tensor_tensor(
                              out=v_aug[:, t, 8 * half:8 * (half + 1), 0:64],
                              in0=ps.rearrange("p (a b) -> p a b", b=64),
                              in1=bv_r[:, 512 * half:512 * (half + 1)].rearrange(
                                  "p (a b) -> p a b", b=64),
                              op=OP.add)
                      else:
                          nc.vector.tensor_copy(
                              out=v_aug[:, t, 8 * half:8 * (half + 1), 0:64],
                              in_=ps.rearrange("p (a b) -> p a b", b=64))

              # A2A(v): shard g = v_aug[:, :, 2g:2g+2, :] (heads 2g, 2g+1)
              v_in_v = v_in[:].rearrange("(g t p x) -> p g t x", p=128, x=2 * 65, g=8)
              for g in range(8):
                  for t in range(NT):
                      nc.sync.dma_start(
                          out=v_in_v[:, g, t],
                          in_=v_aug[:, t, 2 * g:2 * (g + 1)].rearrange("p a b -> p (a b)"))
              nc.gpsimd.collective_compute(
                  "AllToAll", OP.bypass, replica_groups=GROUPS,
                  ins=[v_in[:]], outs=[v_out[:]])

              # loads that overlap the collectives: first FFN weight tiles
              w1_view = w1_d.rearrange("(a p) f -> p a f", p=128)
              w1_pre = []
              for fo in range(2):
                  w1_t = w1s.tile([128, NC_F, 128], BF16, tag="w1t",
                                  name=f"w1pre_{_rep}_{fo}")
                  nc.sync.dma_start(out=w1_t,
                                    in_=w1_view[:, :, 128 * fo:128 * (fo + 1)])
                  w1_pre.append(w1_t)

              # ---- gathered attention operands -> SBUF ----
              # (tag reuse: qT/v_aug buffers are dead once the A2A-in DMAs ran)
              # qkF: [p(2 heads x 64), q/k, bb, jc, t] ; src s supplies
              # jc {2s, 2s+1} of each batch.
              qkF = big.tile([128, 2, 2, NJC, 128], BF16, tag="qT")
              qk_out_v = qk_out[:].rearrange("(s x p b c t) -> p s x b c t",
                                             p=128, s=8, x=2, b=2, c=2)
              for s in range(8):
                  for xq in range(2):
                      for bb in range(2):
                          nc.sync.dma_start(
                              out=qkF[:, xq, bb, 2 * s:2 * (s + 1)].rearrange(
                                  "p a b -> p (a b)"),
                              in_=qk_out_v[:, s, xq, bb].rearrange("p a b -> p (a b)"))
              # vF: [p(j mod 128), bb, jc, h2, 65]
              vF = big.tile([128, 2, NJC, 2, 65], BF16, tag="v_aug")
              v_out_v = v_out[:].rearrange("(s b c p x) -> p s b c x",
                                           p=128, x=2 * 65, s=8, b=2)
              for s in range(8):
                  for bb in range(2):
                      for tc in range(2):
                          nc.sync.dma_start(
                              out=vF[:, bb, 2 * s + tc].rearrange("p a b -> p (a b)"),
                              in_=v_out_v[:, s, bb, tc])

              # ---- attention: 2 batches x 4 i-groups, causal j loop ----
              # attn_all: [p(i mod 128), bb, ic, h2, 64]
              attn_all = big.tile([128, 2, NJC, 2, 64], BF16, tag="attn_all")
              for bb in range(2):
                  for ig in range(NIG):
                      njc = 4 * (ig + 1)       # j-chunks for this i-group
                      avs2 = [av_ps.tile([65, RT], F32, tag="av",
                                         name=f"av_{_rep}_{bb}_{ig}_{k2}")
                              for k2 in range(2)]
                      for m in range(njc):
                          d = m - 4 * ig       # >=0 on the diagonal chunks
                          lo = 128 * d if d >= 0 else 0   # first live i-col
                          st = st_ps.tile([128, 2, RT], F32, tag="st")
                          ex = exs.tile([128, 2, RT], BF16, tag="ex")
                          for hi in range(2):
                              nc.tensor.matmul(
                                  st[:, hi, lo:], qkF[64 * hi:64 * hi + 64, 0, bb, m],
                                  qkF[64 * hi:64 * hi + 64, 1, bb,
                                      4 * ig:4 * (ig + 1)].rearrange(
                                          "p a b -> p (a b)")[:, lo:],
                                  start=True, stop=True,
                                  tile_position=(64 * hi, 0))
                          nc.scalar.activation(out=ex[:, :, lo:], in_=st[:, :, lo:],
                                               func=AF.Exp, scale=SCALE)
                          if d > 0:
                              # i-columns < 128d see no valid j in this chunk
                              nc.vector.memset(ex[:, :, :lo], 0.0)
                          if d >= 0:
                              # triangular mask on the diagonal 128-col block,
                              # broadcast over the head dim via 0-stride AP
                              msl = mask_sb[:, d, lo:lo + 128]
                              mask_b = bass.AP(tensor=msl.tensor, offset=msl.offset,
                                               ap=[msl.ap[0], [0, 2]] + list(msl.ap[1:]))
                              nc.vector.tensor_mul(out=ex[:, :, lo:lo + 128],
                                                   in0=ex[:, :, lo:lo + 128],
                                                   in1=mask_b)
                          for hi in range(2):
                              nc.tensor.matmul(avs2[hi], vF[:, bb, m, hi],
                                               ex[:, hi],
                                               start=(m == 0), stop=(m == njc - 1))
                      for hi in range(2):
                          avs = avs_pool.tile([65, RT], F32, tag="avs")
                          nc.vector.tensor_copy(out=avs, in_=avs2[hi])
                          for i4 in range(NT):
                              pt = mm_ps.tile([128, 128], F32, tag="mm")
                              nc.tensor.transpose(pt[:, 0:65],
                                                  avs[:, 128 * i4:128 * (i4 + 1)],
                                                  ident_f[0:65, 0:65])
                              rec = stats.tile([128, 1], F32, tag="rec")
                              nc.vector.reciprocal(out=rec, in_=pt[:, 64:65])
                              nc.vector.tensor_scalar_mul(
                                  out=attn_all[:, bb, 4 * ig + i4, hi, :],
                                  in0=pt[:, 0:64], scalar1=rec)

              # pre-load the Sqrt set for LN2; hides under the attn A2A
              warm(AF.Sqrt, dep=attn_all[:, 1, NJC - 1, 1, 0:1])

              # A2A(attn): shard g = attn_all[:, :, 2g:2g+2, :, :]
              at_in_v = at_in[:].rearrange("(g b c p x) -> p g b c x",
                                           p=128, x=2 * 64, g=8, b=2)
              for g in range(8):
                  for bb in range(2):
                      for tc in range(2):
                          nc.sync.dma_start(
                              out=at_in_v[:, g, bb, tc],
                              in_=attn_all[:, bb, 2 * g + tc].rearrange("p a b -> p (a b)"))
              nc.gpsimd.collective_compute(
                  "AllToAll", OP.bypass, replica_groups=GROUPS,
                  ins=[at_in[:]], outs=[at_out[:]])

              attn_tok = big.tile([128, NT, C], BF16, tag="attn_all")
              at_out_v = at_out[:].rearrange("(s b c p x) -> p s b c x",
                                             p=128, x=2 * 64, s=8, b=2)
              for s in range(8):
                  for bb in range(2):
                      for tc in range(2):
                          nc.sync.dma_start(
                              out=attn_tok[:, 2 * bb + tc, 128 * s:128 * (s + 1)],
                              in_=at_out_v[:, s, bb, tc])

              # ---- residual + LN2 -> h2, transpose -> h2T ----
              h2 = big.tile([128, NT, C], BF16, tag="h_row")
              h2T = big.tile([128, NC_F, RT], BF16, tag="hT")
              for t in range(NT):
                  nc.vector.tensor_add(out=x_sb[:, t], in0=x_sb[:, t], in1=attn_tok[:, t])
                  layernorm(x_sb[:, t], h2[:, t], g2_r)
                  for fc in range(NC_F):
                      pt = mm_ps.tile([128, 128], BF16, tag="mm")
                      nc.tensor.transpose(pt, h2[:, t, 128 * fc:128 * (fc + 1)], ident_b)
                      nc.vector.tensor_copy(out=h2T[:, fc, 128 * t:128 * (t + 1)], in_=pt)

              # pre-load the Gelu set after LN2's last sqrt; hides under the
              # h2T transposes and first FFN1 matmuls
              warm(AF.Gelu, dep=h2[:, NT - 1, 0:1])

              # ---- FFN1 + gelu -> g1T ----
              g1T = big.tile([128, NF_F, RT], BF16, tag="g1T")
              for fo in range(NF_F):
                  if fo < 2:
                      w1_t = w1_pre[fo]
                  else:
                      w1_t = w1s.tile([128, NC_F, 128], BF16, tag="w1t")
                      nc.sync.dma_start(out=w1_t,
                                        in_=w1_view[:, :, 128 * fo:128 * (fo + 1)])
                  ps = mm_ps.tile([128, RT], F32, tag="mm")
                  for ci in range(NC_F):
                      nc.tensor.matmul(ps, w1_t[:, ci, :], h2T[:, ci, :],
                                       start=(ci == 0), stop=(ci == NC_F - 1))
                  nc.scalar.activation(out=g1T[:, fo, :], in_=ps, func=AF.Gelu,
                                       bias=b1_t[:, fo:fo + 1])

              # ---- FFN2 + residual -> y (W2 streamed in quarters) ----
              out_sb = big.tile([128, NT, C], F32, tag="attn_all")
              w2_view = w2_d.rearrange("(a p) c -> p a c", p=128)
              y_view = y_d.rearrange("(t p) c -> p t c", p=128)
              for q4 in range(4):
                  w2_sb = big.tile([128, NF_F, 256], BF16, tag="w2q", bufs=2)
                  nc.sync.dma_start(out=w2_sb,
                                    in_=w2_view[:, :, 256 * q4:256 * (q4 + 1)])
                  for t in range(NT):
                      ps = mm_ps.tile([128, 256], F32, tag="mm")
                      for fo in range(NF_F):
                          nc.tensor.matmul(ps, g1T[:, fo, 128 * t:128 * (t + 1)],
                                           w2_sb[:, fo, :],
                                           start=(fo == 0), stop=(fo == NF_F - 1))
                      dst = out_sb[:, t, 256 * q4:256 * (q4 + 1)]
                      nc.vector.tensor_tensor(out=dst, in0=ps,
                                              in1=x_sb[:, t, 256 * q4:256 * (q4 + 1)],
                                              op=OP.add)
                      if affine:
                          nc.vector.tensor_add(out=dst, in0=dst,
                                               in1=b2_r[:, 256 * q4:256 * (q4 + 1)])
                      if q4 == 3:
                          nc.sync.dma_start(out=y_view[:, t], in_=out_sb[:, t])

              if _rep < reps - 1:
                  # next rep starts with LN1 sqrt; load its set during FFN2
                  warm(AF.Sqrt, dep=g1T[:, NF_F - 1, 0:1])

    nc.compile()
    return nc


_NC_CACHE = {}


def _get_nc(affine=True):
    if affine not in _NC_CACHE:
        _NC_CACHE[affine] = build_nc(affine=affine)
    return _NC_CACHE[affine]


def _affine_trivial(inputs):
    one = lambda a: np.allclose(np.asarray(a, np.float32), 1.0)
    zero = lambda a: not np.any(np.asarray(a, np.float32))
    return (one(inputs["ln1_g"]) and zero(inputs["ln1_b"])
            and one(inputs["ln2_g"]) and zero(inputs["ln2_b"])
            and zero(inputs["bv"]) and zero(inputs["b2"]))


def _prep_in_maps(inputs):
    x = np.asarray(inputs["x"], np.float32)
    cast_b = lambda a: np.asarray(np.asarray(a, np.float32)).astype(ml_dtypes.bfloat16)
    cast_f = lambda a: np.ascontiguousarray(np.asarray(a, np.float32))
    common = {
        "Wq": cast_b(inputs["Wq"]), "Wk": cast_b(inputs["Wk"]),
        "Wv": cast_b(inputs["Wv"]), "W1": cast_b(inputs["W1"]),
        "W2": cast_b(inputs["W2"]),
        "bq": cast_f(inputs["bq"]), "bk": cast_f(inputs["bk"]),
        "bv": cast_f(inputs["bv"]), "b1": cast_f(inputs["b1"]),
        "b2": cast_f(inputs["b2"]),
        "ln1_g": cast_f(inputs["ln1_g"]), "ln1_b": cast_f(inputs["ln1_b"]),
        "ln2_g": cast_f(inputs["ln2_g"]), "ln2_b": cast_f(inputs["ln2_b"]),
    }
    in_maps = []
    for core in range(N_CORES):
        m = dict(common)
        m["x_loc"] = np.ascontiguousarray(np.concatenate(
            [x[0, BT * core:BT * (core + 1)], x[1, BT * core:BT * (core + 1)]]))
        in_maps.append(m)
    return in_maps


def _assemble(results):
    out = np.empty((B, T, C), np.float32)
    for core in range(N_CORES):
        y = results[core]["y"]
        out[0, BT * core:BT * (core + 1)] = y[:BT]
        out[1, BT * core:BT * (core + 1)] = y[BT:]
    return out


def run_spmd(inputs, **kw):
    """Run on hardware; returns (full_output, BassKernelResults)."""
    in_maps = _prep_in_maps(inputs)
    nc = _get_nc(affine=not _affine_trivial(inputs))
    res = run_bass_kernel_spmd(nc, in_maps, core_ids=list(range(N_CORES)), **kw)
    return _assemble(res.results), res


def kernel(**inputs):
    out, _ = run_spmd(inputs)
    return out
